# revision 1
# baseline (speedup 1.0000x reference)
"""Trainium2 Bass kernel for a dense transformer block (B=8, N=1024, C=768, H=12).

Sharding: data-parallel over batch -- one batch element per NeuronCore (8 cores),
weights replicated, no collectives.

Per-core dataflow (x_b: [1024, 768]), all matmul operands bf16:
  LN1 (token-major, bn_stats; gain/bias folded into qkv_w on host)
  -> transpose to feature-major hT [768, 1024] (bf16)
  -> QKV: q,k feature-major [64, 1024]/head (bf16); v token-major + ones col
  -> per head pair: scores^T [keys, queries] = kT.T @ qT (scale folded in Wq),
     + rel-bias Toeplitz band via identity matmul (bf16), exp -> pt bf16
  -> AV transposed: stationary = pt [keys, 128 queries] (full 128-wide),
     moving = v_aug [keys, 65] -> psum [queries, 65] per head; softmax sums land
     per-partition -> reciprocal + tensor_scalar normalize -> transpose -> aT
  -> proj (feature-major) -> transpose -> +x residual -> LN2 -> MLP -> +residual.

DMAs are batched: one big load per weight tensor, 8 x-tiles, 8 out-tiles.
"""

import os

import numpy as np

B, N, C, H, D = 8, 1024, 768, 12, 64
NT = N // 128   # 8 token tiles
KT = C // 128   # 6 feature tiles
F1 = 4 * C      # 3072
RT = F1 // 128  # 24
W = 2 * N - 1   # 2047 toeplitz band width
EPS = 1e-5

LAST_RESULTS = None

_NC_CACHE = {}


def _build_nc(reps=1):
    from contextlib import ExitStack

    import concourse.bacc as bacc
    import concourse.tile as tile
    from concourse import masks, mybir

    f32 = mybir.dt.float32
    bf16 = mybir.dt.bfloat16

    AF = mybir.ActivationFunctionType
    OP = mybir.AluOpType

    nc = bacc.Bacc(
        "TRN2",
        target_bir_lowering=False,
        debug=False,
        enable_asserts=False,
        num_devices=8,
    )

    x_d = nc.dram_tensor("x", [N, C], f32, kind="ExternalInput").ap()
    wqkv_d = nc.dram_tensor("wqkv_t", [128, KT * 3 * C], bf16, kind="ExternalInput").ap()
    wproj_d = nc.dram_tensor("wproj_t", [128, KT * C], bf16, kind="ExternalInput").ap()
    wfc1_d = nc.dram_tensor("wfc1_t", [128, RT * C], bf16, kind="ExternalInput").ap()
    wfc2_d = nc.dram_tensor("wfc2_t", [128, RT * C], bf16, kind="ExternalInput").ap()
    consts_d = nc.dram_tensor("consts", [128, 54], f32, kind="ExternalInput").ap()
    brows_d = nc.dram_tensor("brows", [2, C], f32, kind="ExternalInput").ap()
    rb_d = nc.dram_tensor("rband", [128, H * W], bf16, kind="ExternalInput").ap()
    out_d = nc.dram_tensor("out", [N, C], f32, kind="ExternalOutput").ap()

    with tile.TileContext(nc) as tc, ExitStack() as ctx:
        # ---------------- kernel-wide pools
        cpool = ctx.enter_context(tc.tile_pool(name="const", bufs=1))
        identb = cpool.tile([128, 128], bf16, tag="identb")
        masks.make_identity(nc, identb[:])
        onesP = cpool.tile([1, 128], f32, tag="onesP")
        nc.any.memset(onesP[:], 1.0)
        # consts layout: 0:18 bqkv, 18:24 bproj, 24:48 bfc1, 48:54 bfc2
        # (DMAs for these issue after the x tiles -- see phase A)
        consts = cpool.tile([128, 54], f32, tag="consts")
        bqkv_sb = consts[:, 0:18]
        bproj_sb = consts[:, 18:24]
        bfc1_sb = consts[:, 24:48]
        bfc2_sb = consts[:, 48:54]
        bv_row = cpool.tile([1, C], f32, tag="bvrow")
        bpr_row = cpool.tile([1, C], f32, tag="bprrow")

        # persistent x (residual stream), f32 [128, 8*768]
        xs = cpool.tile([128, NT * C], f32, tag="xs")

        stat = ctx.enter_context(tc.tile_pool(name="stat", bufs=8))
        # feature-major [128,1024] bf16 chain: hT(6) -> qkT(12) -> aT(6) ->
        # yT(6) -> h2T(6); max concurrent ~19
        chain = ctx.enter_context(tc.tile_pool(name="chain", bufs=1))

        def fm_tile(name):
            return chain.tile([128, N], bf16, tag="fm1024", bufs=12, name=name)

        epsc = cpool.tile([128, 1], f32, tag="eps")
        nc.any.memset(epsc[:], EPS)

        def layernorm_to_bf16(dst_ap, src_ap):
            """dst(bf16) = (src - mean) * rsqrt(var + eps); src [128, 768] f32.

            sqrt on ScalarE (the sqrt table set loads once per phase; the
            identity/copy functions used elsewhere live in every set).
            """
            st6 = stat.tile([128, 12], f32, tag="st6", name="st6")
            nc.vector.bn_stats(st6[:, 0:6], src_ap[:, 0:384])
            nc.vector.bn_stats(st6[:, 6:12], src_ap[:, 384:768])
            mv = stat.tile([128, 2], f32, tag="mv", name="mv")
            nc.vector.bn_aggr(mv[:], st6[:])
            sd = stat.tile([128, 1], f32, tag="sd", name="sd")
            nc.scalar.activation(sd[:], mv[:, 1:2], AF.Sqrt, bias=epsc[:])
            rstd = stat.tile([128, 1], f32, tag="rstd", name="rstd")
            nc.vector.reciprocal(rstd[:], sd[:])
            nmrp = stat.tile([128, 1], f32, tag="nmrp", name="nmrp")
            nc.vector.tensor_mul(nmrp[:], mv[:, 0:1], rstd[:])
            nc.vector.tensor_scalar(
                dst_ap, src_ap, rstd[:], nmrp[:], op0=OP.mult, op1=OP.subtract
            )

        for _rep in range(reps):
            # ---------------- phase A: load x, LN1, transpose -> hT (bf16)
            hT = [fm_tile(f"hT{i}") for i in range(KT)]
            wq_scope = tc.tile_pool(name="wqp", bufs=1)
            wqp = wq_scope.__enter__()
            with (
                tc.tile_pool(name="h1p", bufs=6) as h1p,
                tc.tile_pool(name="psA", bufs=8, space="PSUM") as psA,
            ):
                for t in range(NT):
                    nc.sync.dma_start(
                        xs[:, t * C : (t + 1) * C], x_d[t * 128 : (t + 1) * 128, :]
                    )
                nc.sync.dma_start(consts[:], consts_d[:])
                nc.sync.dma_start(bv_row[:], brows_d[0:1, :])
                nc.sync.dma_start(bpr_row[:], brows_d[1:2, :])
                # weight loads issued after x so x arrives first; q/k weights
                # stream per head pair (hp=0 prefetched here, the rest inside
                # the fused loop) so the first scores are never DMA-gated
                wqkv3 = wqkv_d[:].rearrange("p (ct s) -> p ct s", s=2304)
                wqk_tiles = {}

                def load_wqk(hp):
                    wqk = wqp.tile([128, KT * 256], bf16, tag="wqk", bufs=2,
                                   name=f"wqk{hp}")
                    wqk3 = wqk[:].rearrange("p (ct s) -> p ct s", s=256)
                    nc.sync.dma_start(
                        wqk3[:, :, 0:128], wqkv3[:, :, hp * 128 : (hp + 1) * 128]
                    )
                    nc.sync.dma_start(
                        wqk3[:, :, 128:256],
                        wqkv3[:, :, C + hp * 128 : C + (hp + 1) * 128],
                    )
                    wqk_tiles[hp] = wqk

                load_wqk(0)
                wv = wqp.tile([128, KT * C], bf16, tag="wv")
                nc.sync.dma_start(
                    wv[:].rearrange("p (ct s) -> p ct s", s=C),
                    wqkv3[:, :, 2 * C : 3 * C],
                )
                wp_all = cpool.tile([128, KT * C], bf16, tag="wpall")
                for t in range(NT):
                    h1 = h1p.tile([128, C], bf16, tag="h1", name=f"h1_{t}")
                    layernorm_to_bf16(h1[:], xs[:, t * C : (t + 1) * C])
                    for ct in range(KT):
                        ps = psA.tile([128, 128], bf16, tag="tp", name="psa")
                        nc.tensor.transpose(
                            ps[:], h1[:, ct * 128 : (ct + 1) * 128], identb[:]
                        )
                        if ct % 2:
                            nc.scalar.copy(hT[ct][:, t * 128 : (t + 1) * 128], ps[:])
                        else:
                            nc.vector.tensor_copy(
                                hT[ct][:, t * 128 : (t + 1) * 128], ps[:]
                            )

            # ---------------- phases C+D fused: QKV + attention, per head pair.
            # q/k tiles for pair hp are computed right before its scores so the
            # ACT exp stream starts ~35us earlier; v and the lag-1 AV fill the
            # remaining PE slack under the exp-bound window. One shared PSUM
            # pool (3 x [128,1024] slots, sub-sliced) keeps the budget at 8
            # banks incl. the transpose pool.
            aT = [fm_tile(f"aT{i}") for i in range(KT)]
            vaug = [
                chain.tile([128, H * 65], bf16, tag="vaug", bufs=NT, name=f"vaug{t}")
                for t in range(NT)
            ]
            with (
                tc.tile_pool(name="qkp", bufs=4) as qkp,
                tc.tile_pool(name="rbp", bufs=4) as rbp,
                tc.tile_pool(name="ptp", bufs=40) as ptp,
                tc.tile_pool(name="atokp", bufs=4) as atokp,
                tc.tile_pool(name="psS", bufs=2, space="PSUM") as psS,
                tc.tile_pool(name="psM", bufs=2, space="PSUM") as psM,
                tc.tile_pool(name="psT", bufs=2, space="PSUM") as psT,
            ):

                def emit_qk(wqk, hp, sec):
                    jt = hp + 6 * sec
                    qk = qkp.tile([128, N], bf16, tag="qk", name=f"qk{jt}")
                    for qc in range(2):
                        px = psM.tile([128, 512], f32, tag="pm", name="pxq")
                        ps = px[:]
                        for ct in range(KT):
                            nc.tensor.matmul(
                                ps,
                                wqk[:, ct * 256 + sec * 128 : ct * 256 + (sec + 1) * 128],
                                hT[ct][:, qc * 512 : (qc + 1) * 512],
                                start=(ct == 0),
                                stop=(ct == KT - 1),
                            )
                        nc.vector.tensor_scalar_add(
                            qk[:, qc * 512 : (qc + 1) * 512],
                            ps,
                            bqkv_sb[:, jt : jt + 1],
                        )
                    return qk

                def emit_v(trange):
                    for t in trange:
                        vview = vaug[t][:].rearrange("p (h e) -> p h e", e=65)
                        for vc in range(2):
                            px = psM.tile([128, 512], f32, tag="pm", name="pxv")
                            ps = px[:, 0:384]
                            for ct in range(KT):
                                nc.tensor.matmul(
                                    ps,
                                    hT[ct][:, t * 128 : (t + 1) * 128],
                                    wv[:, ct * C + vc * 384 : ct * C + (vc + 1) * 384],
                                    start=(ct == 0),
                                    stop=False,
                                )
                            nc.tensor.matmul(
                                ps,
                                onesP[:],
                                bv_row[:, vc * 384 : (vc + 1) * 384],
                                start=False,
                                stop=True,
                            )
                            nc.vector.tensor_copy(
                                vview[:, vc * 6 : (vc + 1) * 6, 0:64],
                                ps.rearrange("p (h e) -> p h e", e=64),
                            )
                        nc.any.memset(vview[:, :, 64:65], 1.0)

                def emit_avt_qt(hp, ptiles, qt):
                        px = psM.tile([128, 512], f32, tag="pm", name="pxa")
                        psq = px[:, 0:130]
                        # single accumulation group for the 130-col bank:
                        # (kc=0, even) starts -> bank pending-zero, (kc=0, odd)
                        # overwrites its still-pending cols, rest accumulate
                        for kc in range(NT):
                            for odd in range(2):
                                h = 2 * hp + odd
                                nc.tensor.matmul(
                                    psq[:, odd * 65 : (odd + 1) * 65],
                                    ptiles[kc][odd][:, qt * 128 : (qt + 1) * 128],
                                    vaug[kc][:, h * 65 : (h + 1) * 65],
                                    start=(kc == 0 and odd == 0),
                                    stop=(kc == NT - 1 and odd == 1),
                                )
                        rec = stat.tile([128, 2], f32, tag="rec", name="rec")
                        nc.vector.reciprocal(rec[:, 0:1], psq[:, 64:65])
                        nc.vector.reciprocal(rec[:, 1:2], psq[:, 129:130])
                        atok = atokp.tile([128, 128], bf16, tag="atok", name="atok")
                        nc.vector.tensor_scalar_mul(
                            atok[:, 0:64], psq[:, 0:64], rec[:, 0:1]
                        )
                        nc.vector.tensor_scalar_mul(
                            atok[:, 64:128], psq[:, 65:129], rec[:, 1:2]
                        )
                        pst = psT.tile([128, 128], bf16, tag="pst", name="pst")
                        nc.tensor.transpose(pst[:], atok[:], identb[:])
                        nc.vector.tensor_copy(
                            aT[hp][:, qt * 128 : (qt + 1) * 128], pst[:]
                        )

                prev_ptiles = None
                for hp in range(KT):
                    if hp + 1 < KT:
                        load_wqk(hp + 1)
                    if hp == 4:
                        # proj weights deferred here so the transfer does not
                        # block the streaming qk/rband loads
                        nc.sync.dma_start(wp_all[:], wproj_d[:])
                    if hp == 0:
                        wqk = wqk_tiles.pop(0)
                        qkq = emit_qk(wqk, 0, 0)
                        qkk = emit_qk(wqk, 0, 1)
                    rbs = []
                    for odd in range(2):
                        rbh = rbp.tile([128, W], bf16, tag="rb", name=f"rb{2*hp+odd}")
                        nc.sync.dma_start(
                            rbh[:],
                            rb_d[:, (2 * hp + odd) * W : (2 * hp + odd + 1) * W],
                        )
                        rbs.append(rbh)
                    ptiles = [[None] * 2 for _ in range(NT)]
                    for kc in range(NT):
                        for odd in range(2):
                            ro = odd * 64
                            ps = psS.tile([128, 1024], f32, tag="ps", name="pxs")
                            for qc in range(2):
                                nc.tensor.matmul(
                                    ps[:, qc * 512 : (qc + 1) * 512],
                                    qkk[ro : ro + 64, kc * 128 : (kc + 1) * 128],
                                    qkq[ro : ro + 64, qc * 512 : (qc + 1) * 512],
                                    start=True,
                                    stop=True,
                                )
                            pt = ptp.tile([128, 1024], bf16, tag="pt", name="pt")
                            nc.scalar.activation(pt[:], ps[:], AF.Exp)
                            # rel-bias: pt *= exp(band) window (host-precomputed;
                            # qc=0/1 windows are adjacent -> one 1024-wide TT)
                            off = 1023 - kc * 128
                            nc.vector.tensor_mul(
                                pt[:], pt[:], rbs[odd][:, off : off + 1024]
                            )
                            ptiles[kc][odd] = pt
                        # spread next pair's q/k and the v tiles between kc
                        # steps: PE stays busy while ACT drains the exp
                        # backlog, and the AV-only interlude at loop end
                        # shrinks
                        if hp + 1 < KT:
                            if kc == 2:
                                wqk_n = wqk_tiles.pop(hp + 1)
                                qkq_n = emit_qk(wqk_n, hp + 1, 0)
                            elif kc == 4:
                                qkk_n = emit_qk(wqk_n, hp + 1, 1)
                        if hp < 2 and kc % 2 == 1:
                            emit_v(range(4 * hp + kc // 2, 4 * hp + kc // 2 + 1))
                        # retire the first AV query tiles of the previous pair
                        # inside the scores stream so the end-of-pair PE block
                        # stays below the exp backlog depth
                        if hp >= 1 and kc >= 5:
                            emit_avt_qt(hp - 1, prev_ptiles, kc - 5)
                    if hp >= 1:
                        for qt in range(3, NT):
                            emit_avt_qt(hp - 1, prev_ptiles, qt)
                    prev_ptiles = ptiles
                    if hp + 1 < KT:
                        qkq, qkk = qkq_n, qkk_n
                for qt in range(NT):
                    emit_avt_qt(KT - 1, prev_ptiles, qt)

            wq_scope.__exit__(None, None, None)

            # ---------------- phases E/F/H: proj+residual+LN2 then MLP per
            # half; proj/LN2 of half 1 is emitted between MLP-0's matmuls and
            # its stores so its DVE/ACT chain hides under MLP-0 PE work.
            h2T = [fm_tile(f"h2T{i}") for i in range(KT)]
            w2_scope = tc.tile_pool(name="w2p", bufs=1)
            w2p = w2_scope.__enter__()
            w1_all = w2p.tile([128, RT * C], bf16, tag="w1all")
            w2_all = w2p.tile([128, RT * C], bf16, tag="w2all")
            # chunked so the first r-blocks land before the MLP starts instead
            # of gating it behind one long transfer
            for r0 in range(0, RT, 6):
                cols = slice(r0 * C, (r0 + 6) * C)
                nc.sync.dma_start(w1_all[:, cols], wfc1_d[:, cols])
                nc.sync.dma_start(w2_all[:, cols], wfc2_d[:, cols])
            with (
                tc.tile_pool(name="h2p", bufs=6) as h2p,
                tc.tile_pool(name="grp", bufs=4) as grp,
                tc.tile_pool(name="o2p", bufs=13) as o2p,
                tc.tile_pool(name="obp", bufs=4) as obp,
            ):

                def emit_projF(qc):
                    # proj token-major (stationary=aT chunk, moving=wproj rows),
                    # bias via rank-1 ones x bproj_row, + residual + LN2
                    with tc.tile_pool(name="psP", bufs=2, space="PSUM") as psP:
                        for i in range(4):
                            t = qc * 4 + i
                            pp = psP.tile([128, C], f32, tag="pp", name=f"pp{t}")
                            for c0 in range(0, C, 512):
                                c1 = min(c0 + 512, C)
                                for hp in range(KT):
                                    nc.tensor.matmul(
                                        pp[:, c0:c1],
                                        aT[hp][:, t * 128 : (t + 1) * 128],
                                        wp_all[:, hp * C + c0 : hp * C + c1],
                                        start=(hp == 0),
                                        stop=False,
                                    )
                                nc.tensor.matmul(
                                    pp[:, c0:c1],
                                    onesP[:],
                                    bpr_row[:, c0:c1],
                                    start=False,
                                    stop=True,
                                )
                            nc.vector.tensor_add(
                                xs[:, t * C : (t + 1) * C],
                                xs[:, t * C : (t + 1) * C],
                                pp[:],
                            )
                            h2 = h2p.tile([128, C], bf16, tag="h2", name=f"h2_{t}")
                            layernorm_to_bf16(h2[:], xs[:, t * C : (t + 1) * C])
                            for ct in range(KT):
                                ps = psP.tile(
                                    [128, 128], bf16, tag="tp", bufs=4, name="psf2"
                                )
                                nc.tensor.transpose(
                                    ps[:], h2[:, ct * 128 : (ct + 1) * 128], identb[:]
                                )
                                nc.scalar.copy(
                                    h2T[ct][:, t * 128 : (t + 1) * 128], ps[:]
                                )

                def emit_mlp(qc):
                    o2 = []
                    with tc.tile_pool(name="psO", bufs=6, space="PSUM") as ps_o:
                        pso = [
                            ps_o.tile([128, 512], f32, tag="pso", name=f"pso{qc}_{i}")
                            for i in range(KT)
                        ]
                        with tc.tile_pool(name="psG2", bufs=2, space="PSUM") as ps_g:
                            for r in range(RT):
                                psg = ps_g.tile([128, 512], f32, tag="psg", name="psg")
                                for ct in range(KT):
                                    nc.tensor.matmul(
                                        psg[:],
                                        w1_all[:, r * C + ct * 128 : r * C + (ct + 1) * 128],
                                        h2T[ct][:, qc * 512 : (qc + 1) * 512],
                                        start=(ct == 0),
                                        stop=(ct == KT - 1),
                                    )
                                gr = grp.tile([128, 512], bf16, tag="gr", name="gr")
                                nc.scalar.activation(
                                    gr[:], psg[:], AF.Gelu, bias=bfc1_sb[:, r : r + 1]
                                )
                                for co in range(KT):
                                    nc.tensor.matmul(
                                        pso[co][:],
                                        w2_all[:, r * C + co * 128 : r * C + (co + 1) * 128],
                                        gr[:],
                                        start=(r == 0),
                                        stop=(r == RT - 1),
                                    )
                        for co in range(KT):
                            o2t = o2p.tile(
                                [128, 512], bf16, tag="o2", name=f"o2_{qc}_{co}"
                            )
                            # bias-add on ScalarE (idle after gelu) so the
                            # store chain's DVE adds run in parallel
                            nc.scalar.activation(
                                o2t[:], pso[co][:], AF.Identity,
                                bias=bfc2_sb[:, co : co + 1],
                            )
                            o2.append(o2t)
                    return o2

                def emit_stores(qc, o2):
                    with tc.tile_pool(name="psH", bufs=2, space="PSUM") as psH:
                        for t4 in range(4):
                            t = qc * 4 + t4
                            ob = obp.tile([128, C], f32, tag="ob", name="ob")
                            for co in range(KT):
                                ps = psH.tile([128, 128], bf16, tag="tp", name="psh")
                                nc.tensor.transpose(
                                    ps[:], o2[co][:, t4 * 128 : (t4 + 1) * 128], identb[:]
                                )
                                nc.vector.tensor_add(
                                    ob[:, co * 128 : (co + 1) * 128],
                                    xs[:, t * C + co * 128 : t * C + (co + 1) * 128],
                                    ps[:],
                                )
                            nc.sync.dma_start(out_d[t * 128 : (t + 1) * 128, :], ob[:])

                emit_projF(0)
                o2_0 = emit_mlp(0)
                emit_projF(1)
                emit_stores(0, o2_0)
                o2_1 = emit_mlp(1)
                emit_stores(1, o2_1)
            w2_scope.__exit__(None, None, None)

    nc.compile()
    return nc


def _get_nc(reps=1):
    key = f"nc{reps}"
    if key not in _NC_CACHE:
        _NC_CACHE[key] = _build_nc(reps)
    return _NC_CACHE[key]


def _host_prep(inputs):
    import ml_dtypes

    bf = ml_dtypes.bfloat16
    inp = {k: np.asarray(v) for k, v in inputs.items()}
    x = np.ascontiguousarray(inp["x"], dtype=np.float32)  # [8, 1024, 768]
    g1 = inp["ln1_g"].astype(np.float64)
    b1 = inp["ln1_b"].astype(np.float64)
    qkv_w = inp["qkv_w"].astype(np.float64)  # [2304, 768]
    Ws = qkv_w.copy()
    Ws[:C] *= D ** (-0.5)  # fold attention scale into Wq
    wqkvT = (Ws * g1[None, :]).T  # [768, 2304]
    # partition-major [128, KT*2304]: wqkv_t[p, ct*2304 + c] = wqkvT[ct*128+p, c]
    wqkv_t = np.ascontiguousarray(
        wqkvT.reshape(KT, 128, 3 * C).transpose(1, 0, 2).reshape(128, KT * 3 * C)
    ).astype(bf)
    bqkv = (Ws @ b1).astype(np.float32)  # [2304]

    wprojT = inp["proj_w"].astype(np.float32).T  # [768, 768]
    wproj_t = np.ascontiguousarray(
        wprojT.reshape(KT, 128, C).transpose(1, 0, 2).reshape(128, KT * C)
    ).astype(bf)
    bproj = inp["proj_b"].astype(np.float32)

    g2 = inp["ln2_g"].astype(np.float64)
    b2 = inp["ln2_b"].astype(np.float64)
    fc1_w = inp["fc1_w"].astype(np.float64)  # [3072, 768]
    wfc1T = (fc1_w * g2[None, :]).T  # [768, 3072]
    # [128, RT*C]: wfc1_t[p, r*C + ct*128+j] = wfc1T[ct*128+p, r*128+j]
    wfc1_t = np.ascontiguousarray(
        wfc1T.reshape(KT, 128, RT, 128).transpose(1, 2, 0, 3).reshape(128, RT * C)
    ).astype(bf)
    bfc1 = (fc1_w @ b2 + inp["fc1_b"].astype(np.float64)).astype(np.float32)  # [3072]
    wfc2T = inp["fc2_w"].astype(np.float32).T  # [3072, 768]
    # [128, RT*C]: wfc2_t[p, r*C + c] = wfc2T[r*128+p, c]
    wfc2_t = np.ascontiguousarray(
        wfc2T.reshape(RT, 128, C).transpose(1, 0, 2).reshape(128, RT * C)
    ).astype(bf)
    bfc2 = inp["fc2_b"].astype(np.float32)

    # consts [128, 54]: bqkv (p-major 18), bproj 6, bfc1 24, bfc2 6
    consts = np.zeros((128, 54), np.float32)
    consts[:, 0:18] = bqkv.reshape(18, 128).T
    consts[:, 18:24] = bproj.reshape(6, 128).T
    consts[:, 24:48] = bfc1.reshape(24, 128).T
    consts[:, 48:54] = bfc2.reshape(6, 128).T
    brows = np.stack([bqkv[2 * C :], bproj]).astype(np.float32)  # [2, C]

    # multiplicative rel-bias toeplitz band, [128, H*W]: rband[p, h*W + w] =
    #   exp(rel_table[clip(p + 1087 - w, 0, 128), h]); applied as pt *= band
    tab = np.exp(inp["rel_table"].astype(np.float64)).astype(np.float32)  # [129, 12]
    p_i = np.arange(128)
    w_i = np.arange(W)
    idx = np.clip(p_i[:, None] + (N + 63) - w_i[None, :], 0, 2 * 64)
    rband = np.ascontiguousarray(
        tab[idx, :].transpose(0, 2, 1).reshape(128, H * W)
    ).astype(bf)

    shared = {
        "wqkv_t": wqkv_t,
        "wproj_t": wproj_t,
        "wfc1_t": wfc1_t,
        "wfc2_t": wfc2_t,
        "consts": consts,
        "brows": brows,
        "rband": rband,
    }
    in_maps = [{"x": np.ascontiguousarray(x[c]), **shared} for c in range(B)]
    return in_maps


def _make_runner(reps=1):
    import jax
    from jax.experimental.shard_map import shard_map
    from jax.sharding import Mesh, NamedSharding, PartitionSpec

    from concourse import bass2jax, mybir

    nc = _get_nc(reps)
    bass2jax.install_neuronx_cc_hook()

    partition_name = nc.partition_id_tensor.name if nc.partition_id_tensor else None
    in_names, out_names, out_avals, zero_outs = [], [], [], []
    for alloc in nc.m.functions[0].allocations:
        if not isinstance(alloc, mybir.MemoryLocationSet):
            continue
        name = alloc.memorylocations[0].name
        if alloc.kind == "ExternalInput":
            if name != partition_name:
                in_names.append(name)
        elif alloc.kind == "ExternalOutput":
            out_names.append(name)
            shape = tuple(alloc.tensor_shape)
            dtype = mybir.dt.np(alloc.dtype)
            out_avals.append(jax.core.ShapedArray(shape, dtype))
            zero_outs.append(np.zeros(shape, dtype))
    n_params = len(in_names)
    all_names = tuple(in_names) + tuple(out_names)
    if partition_name is not None:
        all_names = all_names + (partition_name,)
    donate = tuple(range(n_params, n_params + len(out_names)))

    def _body(*args):
        operands = list(args)
        if partition_name is not None:
            operands.append(bass2jax.partition_id_tensor())
        outs = bass2jax._bass_exec_p.bind(
            *operands,
            out_avals=tuple(out_avals),
            in_names=all_names,
            out_names=tuple(out_names),
            lowering_input_output_aliases=(),
            sim_require_finite=True,
            sim_require_nnan=True,
            nc=nc,
        )
        return tuple(outs)

    def _body_k(k):
        def body(*args):
            ins = list(args[:n_params])
            outs = list(args[n_params:])
            for _ in range(k):
                outs = list(_body(*ins, *outs))
            return tuple(outs)

        return body

    devices = jax.devices()[:B]
    mesh = Mesh(np.asarray(devices), ("core",))
    in_specs = (PartitionSpec("core"),) * (n_params + len(out_names))
    out_specs = (PartitionSpec("core"),) * len(out_names)

    def make_fn(k):
        return jax.jit(
            shard_map(
                _body_k(k),
                mesh=mesh,
                in_specs=in_specs,
                out_specs=out_specs,
                check_rep=False,
            ),
            donate_argnums=donate,
            keep_unused=True,
        )

    sharding = NamedSharding(mesh, PartitionSpec("core"))
    return make_fn, in_names, out_names, zero_outs, sharding


def _get_runner(reps=1):
    key = f"runner{reps}"
    if key not in _NC_CACHE:
        _NC_CACHE[key] = _make_runner(reps)
    return _NC_CACHE[key]


LAST_BENCH = None


def kernel(**inputs):
    global LAST_BENCH
    import time

    import jax

    make_fn, in_names, out_names, zero_outs, sharding = _get_runner()
    in_maps = _host_prep(inputs)
    concat_in = [
        np.concatenate([np.asarray(in_maps[c][n]) for c in range(B)], axis=0)
        for n in in_names
    ]
    concat_zeros = [
        np.zeros((B * z.shape[0], *z.shape[1:]), z.dtype) for z in zero_outs
    ]
    fn1 = make_fn(1)
    dev_in = [jax.device_put(a, sharding) for a in concat_in]
    outs = fn1(*dev_in, *concat_zeros)
    jax.block_until_ready(outs)
    result = np.asarray(outs[0]).reshape(B, N, C).astype(np.float32)

    iters = int(os.environ.get("BENCH_ITERS", "0"))
    if iters > 0:
        o = fn1(*dev_in, *outs)  # warm
        jax.block_until_ready(o)
        times = []
        for _ in range(iters):
            t0 = time.perf_counter()
            o = fn1(*dev_in, *o)
            jax.block_until_ready(o)
            times.append(time.perf_counter() - t0)
        overhead = _bench_overhead()
        t_min = float(np.min(times))
        t_med = float(np.median(times))
        LAST_BENCH = {
            "per_iter_ns": max(t_min - overhead, 0.0) * 1e9,
            "call_min_ns": t_min * 1e9,
            "call_med_ns": t_med * 1e9,
            "overhead_ns": overhead * 1e9,
            "iters": iters,
        }
    return result


def _bench_overhead():
    """Per-call dispatch overhead, measured with a trivial 1-DMA kernel."""
    import time

    import jax
    from jax.experimental.shard_map import shard_map
    from jax.sharding import Mesh, PartitionSpec

    import concourse.bacc as bacc
    import concourse.tile as tile
    from concourse import bass2jax, mybir

    if "tiny" not in _NC_CACHE:
        f32 = mybir.dt.float32
        nc = bacc.Bacc(
            "TRN2",
            target_bir_lowering=False,
            debug=False,
            enable_asserts=False,
            num_devices=8,
        )
        xi = nc.dram_tensor("ti", [128, 128], f32, kind="ExternalInput").ap()
        xo = nc.dram_tensor("to", [128, 128], f32, kind="ExternalOutput").ap()
        with tile.TileContext(nc) as tc:
            with tc.tile_pool(name="p", bufs=1) as p:
                t = p.tile([128, 128], f32, tag="t", name="t")
                nc.sync.dma_start(t[:], xi[:])
                nc.sync.dma_start(xo[:], t[:])
        nc.compile()

        partition_name = nc.partition_id_tensor.name if nc.partition_id_tensor else None
        all_names = ["ti", "to"]
        if partition_name is not None:
            all_names.append(partition_name)
        out_avals = [jax.core.ShapedArray((128, 128), np.float32)]

        def _tbody(*args):
            operands = list(args)
            if partition_name is not None:
                operands.append(bass2jax.partition_id_tensor())
            return tuple(
                bass2jax._bass_exec_p.bind(
                    *operands,
                    out_avals=tuple(out_avals),
                    in_names=tuple(all_names),
                    out_names=("to",),
                    lowering_input_output_aliases=(),
                    sim_require_finite=True,
                    sim_require_nnan=True,
                    nc=nc,
                )
            )

        devices = jax.devices()[:B]
        mesh = Mesh(np.asarray(devices), ("core",))
        tfn = jax.jit(
            shard_map(
                _tbody,
                mesh=mesh,
                in_specs=(PartitionSpec("core"),) * 2,
                out_specs=(PartitionSpec("core"),),
                check_rep=False,
            ),
            donate_argnums=(1,),
            keep_unused=True,
        )
        _NC_CACHE["tiny"] = tfn

    tfn = _NC_CACHE["tiny"]
    ti = np.zeros((B * 128, 128), np.float32)
    o = tfn(ti, np.zeros((B * 128, 128), np.float32))
    jax.block_until_ready(o)
    times = []
    for _ in range(30):
        t0 = time.perf_counter()
        o = tfn(ti, *([o] if not isinstance(o, tuple) else list(o)))
        jax.block_until_ready(o)
        times.append(time.perf_counter() - t0)
    return float(np.min(times))



# revision 2
# speedup vs baseline: 1.0511x; 1.0511x over previous
"""Trainium2 Bass kernel for a dense transformer block (B=8, N=1024, C=768, H=12).

Sharding: data-parallel over batch -- one batch element per NeuronCore (8 cores),
weights replicated, no collectives.

v2: attention matmuls in fp8e4m3 with DoubleRow (double-pumped) mode:
  - hT stored fp8 in ct-pair layout [128, 2, 1024] -> QKV matmuls DoubleRow
  - q/k stored fp8 (x8 scale), scores matmul fp8 (contraction 64)
  - exp compensates scales via activation scale=1/512
  - v/pt/AV stay bf16 (keeps DVE 2x for the band multiply)
  - aT stored fp8 (x8) in hp-pair layout -> proj matmul DoubleRow (w x16)
  - bias rank-1 matmuls in bf16 (were fp32: 4 cycles/row)
  - MLP stays bf16 for error headroom.
"""

import os

import numpy as np

B, N, C, H, D = 8, 1024, 768, 12, 64
NT = N // 128   # 8 token tiles
KT = C // 128   # 6 feature tiles
F1 = 4 * C      # 3072
RT = F1 // 128  # 24
W = 2 * N - 1   # 2047 toeplitz band width
EPS = 1e-5

SW = 16.0       # fp8 weight pre-scale
SQK = 8.0       # fp8 q/k storage scale
SA = 8.0        # fp8 aT storage scale

LAST_RESULTS = None

_NC_CACHE = {}


def _build_nc(reps=1):
    from contextlib import ExitStack

    import concourse.bacc as bacc
    import concourse.tile as tile
    from concourse import masks, mybir

    f32 = mybir.dt.float32
    bf16 = mybir.dt.bfloat16
    f8 = mybir.dt.float8e4

    AF = mybir.ActivationFunctionType
    OP = mybir.AluOpType
    DR = mybir.MatmulPerfMode.DoubleRow

    nc = bacc.Bacc(
        "TRN2",
        target_bir_lowering=False,
        debug=False,
        enable_asserts=False,
        num_devices=8,
    )

    x_d = nc.dram_tensor("x", [N, C], f32, kind="ExternalInput").ap()
    wqkv_d = nc.dram_tensor("wqkv_t", [128, KT * 3 * C], f8, kind="ExternalInput").ap()
    wproj_d = nc.dram_tensor("wproj_t", [128, KT * C], f8, kind="ExternalInput").ap()
    wfc1_d = nc.dram_tensor("wfc1_t", [128, RT * C], bf16, kind="ExternalInput").ap()
    wfc2_d = nc.dram_tensor("wfc2_t", [128, RT * C], bf16, kind="ExternalInput").ap()
    consts_d = nc.dram_tensor("consts", [128, 54], f32, kind="ExternalInput").ap()
    brows_d = nc.dram_tensor("brows", [2, C], bf16, kind="ExternalInput").ap()
    rb_d = nc.dram_tensor("rband", [128, H * W], bf16, kind="ExternalInput").ap()
    out_d = nc.dram_tensor("out", [N, C], f32, kind="ExternalOutput").ap()

    with tile.TileContext(nc) as tc, ExitStack() as ctx:
        # ---------------- kernel-wide pools
        cpool = ctx.enter_context(tc.tile_pool(name="const", bufs=1))
        identb = cpool.tile([128, 128], bf16, tag="identb")
        masks.make_identity(nc, identb[:])
        onesP = cpool.tile([1, 128], bf16, tag="onesP")
        nc.any.memset(onesP[:], 1.0)
        # consts layout: 0:12 bqkv(qk, x8), 18:24 bproj(unused; brows carries),
        # 24:48 bfc1, 48:54 bfc2
        consts = cpool.tile([128, 54], f32, tag="consts")
        bqkv_sb = consts[:, 0:18]
        bfc1_sb = consts[:, 24:48]
        bfc2_sb = consts[:, 48:54]
        bv_row = cpool.tile([1, C], bf16, tag="bvrow")
        bpr_row = cpool.tile([1, C], bf16, tag="bprrow")

        # persistent x (residual stream), f32 [128, 8*768]
        xs = cpool.tile([128, NT * C], f32, tag="xs")

        stat = ctx.enter_context(tc.tile_pool(name="stat", bufs=8))
        chain = ctx.enter_context(tc.tile_pool(name="chain", bufs=1))

        def fm_tile(name):
            return chain.tile([128, N], bf16, tag="fm1024", bufs=6, name=name)

        def pair_tile(name):
            # fp8 ct-pair layout: [128, 2*1024]
            return chain.tile([128, 2 * N], f8, tag="pair2048", bufs=7, name=name)

        epsc = cpool.tile([128, 1], f32, tag="eps")
        nc.any.memset(epsc[:], EPS)

        def layernorm_to_bf16(dst_ap, src_ap):
            """dst(bf16) = (src - mean) * rsqrt(var + eps); src [128, 768] f32."""
            st6 = stat.tile([128, 12], f32, tag="st6", name="st6")
            nc.vector.bn_stats(st6[:, 0:6], src_ap[:, 0:384])
            nc.vector.bn_stats(st6[:, 6:12], src_ap[:, 384:768])
            mv = stat.tile([128, 2], f32, tag="mv", name="mv")
            nc.vector.bn_aggr(mv[:], st6[:])
            sd = stat.tile([128, 1], f32, tag="sd", name="sd")
            nc.scalar.activation(sd[:], mv[:, 1:2], AF.Sqrt, bias=epsc[:])
            rstd = stat.tile([128, 1], f32, tag="rstd", name="rstd")
            nc.vector.reciprocal(rstd[:], sd[:])
            nmrp = stat.tile([128, 1], f32, tag="nmrp", name="nmrp")
            nc.vector.tensor_mul(nmrp[:], mv[:, 0:1], rstd[:])
            nc.vector.tensor_scalar(
                dst_ap, src_ap, rstd[:], nmrp[:], op0=OP.mult, op1=OP.subtract
            )

        for _rep in range(reps):
            # ---------------- phase A: load x, LN1, transpose -> hTp (fp8 pairs)
            hTp = [pair_tile(f"hTp{i}") for i in range(KT // 2)]

            def hT_view(i):
                return hTp[i][:].rearrange("p (two n) -> p two n", two=2)

            wq_scope = tc.tile_pool(name="wqp", bufs=1)
            wqp = wq_scope.__enter__()
            with (
                tc.tile_pool(name="h1p", bufs=6) as h1p,
                tc.tile_pool(name="psA", bufs=8, space="PSUM") as psA,
            ):
                for t in range(NT):
                    nc.sync.dma_start(
                        xs[:, t * C : (t + 1) * C], x_d[t * 128 : (t + 1) * 128, :]
                    )
                nc.sync.dma_start(consts[:], consts_d[:])
                nc.sync.dma_start(bv_row[:], brows_d[0:1, :])
                nc.sync.dma_start(bpr_row[:], brows_d[1:2, :])
                wqkv3 = wqkv_d[:].rearrange("p (ct s) -> p ct s", s=2304)
                wqk_tiles = {}

                def load_wqk(hp):
                    wqk = wqp.tile([128, KT * 256], f8, tag="wqk", bufs=2,
                                   name=f"wqk{hp}")
                    wqk3 = wqk[:].rearrange("p (ct s) -> p ct s", s=256)
                    nc.sync.dma_start(
                        wqk3[:, :, 0:128], wqkv3[:, :, hp * 128 : (hp + 1) * 128]
                    )
                    nc.sync.dma_start(
                        wqk3[:, :, 128:256],
                        wqkv3[:, :, C + hp * 128 : C + (hp + 1) * 128],
                    )
                    wqk_tiles[hp] = wqk

                load_wqk(0)
                wv = wqp.tile([128, KT * C], f8, tag="wv")
                nc.sync.dma_start(
                    wv[:].rearrange("p (ct s) -> p ct s", s=C),
                    wqkv3[:, :, 2 * C : 3 * C],
                )
                wp_all = cpool.tile([128, KT * C], f8, tag="wpall")
                for t in range(NT):
                    h1 = h1p.tile([128, C], bf16, tag="h1", name=f"h1_{t}")
                    layernorm_to_bf16(h1[:], xs[:, t * C : (t + 1) * C])
                    for ct in range(KT):
                        ps = psA.tile([128, 128], bf16, tag="tp", name="psa")
                        nc.tensor.transpose(
                            ps[:], h1[:, ct * 128 : (ct + 1) * 128], identb[:]
                        )
                        dst = hTp[ct // 2][
                            :, (ct % 2) * N + t * 128 : (ct % 2) * N + (t + 1) * 128
                        ]
                        if ct % 2:
                            nc.scalar.copy(dst, ps[:])
                        else:
                            nc.vector.tensor_copy(dst, ps[:])

            # ---------------- phases C+D fused: QKV + attention, per head pair.
            aTp = [pair_tile(f"aTp{i}") for i in range(KT // 2)]
            vaug = [
                chain.tile([128, H * 65], bf16, tag="vaug", bufs=NT, name=f"vaug{t}")
                for t in range(NT)
            ]
            with (
                tc.tile_pool(name="qkp", bufs=4) as qkp,
                tc.tile_pool(name="rbp", bufs=4) as rbp,
                tc.tile_pool(name="ptp", bufs=40) as ptp,
                tc.tile_pool(name="atokp", bufs=4) as atokp,
                tc.tile_pool(name="psS", bufs=2, space="PSUM") as psS,
                tc.tile_pool(name="psM", bufs=2, space="PSUM") as psM,
                tc.tile_pool(name="psT", bufs=2, space="PSUM") as psT,
            ):

                def emit_qk(wqk, hp, sec):
                    # out fp8 q or k tile [128, N] = 8*(raw + bias)
                    jt = hp + 6 * sec
                    qk = qkp.tile([128, N], f8, tag="qk", name=f"qk{jt}")
                    wqk3 = wqk[:].rearrange("p (ct s) -> p ct s", s=256)
                    for qc in range(2):
                        px = psM.tile([128, 512], f32, tag="pm", name="pxq")
                        ps = px[:]
                        for cp in range(KT // 2):
                            nc.tensor.matmul(
                                ps,
                                wqk3[
                                    :, 2 * cp : 2 * cp + 2,
                                    sec * 128 : (sec + 1) * 128,
                                ],
                                hT_view(cp)[:, :, qc * 512 : (qc + 1) * 512],
                                start=(cp == 0),
                                stop=(cp == KT // 2 - 1),
                                perf_mode=DR,
                            )
                        # qk_f8 = ps * (SQK/SW) + bias8 (bias pre-scaled x8 host)
                        nc.vector.tensor_scalar(
                            qk[:, qc * 512 : (qc + 1) * 512],
                            ps,
                            SQK / SW,
                            bqkv_sb[:, jt : jt + 1],
                            op0=OP.mult,
                            op1=OP.add,
                        )
                    return qk

                def emit_v(trange):
                    for t in trange:
                        vview = vaug[t][:].rearrange("p (h e) -> p h e", e=65)
                        for vc in range(2):
                            px = psM.tile([128, 512], f32, tag="pm", name="pxv")
                            ps = px[:, 0:384]
                            wv3 = wv[:].rearrange("p (ct s) -> p ct s", s=C)
                            for cp in range(KT // 2):
                                nc.tensor.matmul(
                                    ps,
                                    hT_view(cp)[:, :, t * 128 : (t + 1) * 128],
                                    wv3[
                                        :, 2 * cp : 2 * cp + 2,
                                        vc * 384 : (vc + 1) * 384,
                                    ],
                                    start=(cp == 0),
                                    stop=False,
                                    perf_mode=DR,
                                )
                            nc.tensor.matmul(
                                ps,
                                onesP[:],
                                bv_row[:, vc * 384 : (vc + 1) * 384],
                                start=False,
                                stop=True,
                            )
                            # v = ps / SW  (bias row pre-scaled x16 host)
                            nc.vector.tensor_scalar(
                                vview[:, vc * 6 : (vc + 1) * 6, 0:64],
                                ps.rearrange("p (h e) -> p h e", e=64),
                                1.0 / SW,
                                None,
                                op0=OP.mult,
                            )
                        nc.any.memset(vview[:, :, 64:65], 1.0)

                def emit_avt_qt(hp, ptiles, qt):
                        px = psM.tile([128, 512], f32, tag="pm", name="pxa")
                        psq = px[:, 0:130]
                        for kc in range(NT):
                            for odd in range(2):
                                h = 2 * hp + odd
                                nc.tensor.matmul(
                                    psq[:, odd * 65 : (odd + 1) * 65],
                                    ptiles[kc][odd][:, qt * 128 : (qt + 1) * 128],
                                    vaug[kc][:, h * 65 : (h + 1) * 65],
                                    start=(kc == 0 and odd == 0),
                                    stop=(kc == NT - 1 and odd == 1),
                                )
                        rec = stat.tile([128, 2], f32, tag="rec", name="rec")
                        nc.vector.reciprocal(rec[:, 0:1], psq[:, 64:65])
                        nc.vector.reciprocal(rec[:, 1:2], psq[:, 129:130])
                        atok = atokp.tile([128, 128], bf16, tag="atok", name="atok")
                        # atok = (psq * rec) * SA  (fp8 storage scale)
                        nc.vector.tensor_scalar(
                            atok[:, 0:64], psq[:, 0:64], rec[:, 0:1], SA,
                            op0=OP.mult, op1=OP.mult,
                        )
                        nc.vector.tensor_scalar(
                            atok[:, 64:128], psq[:, 65:129], rec[:, 1:2], SA,
                            op0=OP.mult, op1=OP.mult,
                        )
                        pst = psT.tile([128, 128], bf16, tag="pst", name="pst")
                        nc.tensor.transpose(pst[:], atok[:], identb[:])
                        dst = aTp[hp // 2][
                            :, (hp % 2) * N + qt * 128 : (hp % 2) * N + (qt + 1) * 128
                        ]
                        nc.vector.tensor_copy(dst, pst[:])

                prev_ptiles = None
                for hp in range(KT):
                    if hp + 1 < KT:
                        load_wqk(hp + 1)
                    if hp == 4:
                        nc.sync.dma_start(wp_all[:], wproj_d[:])
                    if hp == 0:
                        wqk = wqk_tiles.pop(0)
                        qkq = emit_qk(wqk, 0, 0)
                        qkk = emit_qk(wqk, 0, 1)
                    rbs = []
                    for odd in range(2):
                        rbh = rbp.tile([128, W], bf16, tag="rb", name=f"rb{2*hp+odd}")
                        nc.sync.dma_start(
                            rbh[:],
                            rb_d[:, (2 * hp + odd) * W : (2 * hp + odd + 1) * W],
                        )
                        rbs.append(rbh)
                    ptiles = [[None] * 2 for _ in range(NT)]
                    for kc in range(NT):
                        for odd in range(2):
                            ro = odd * 64
                            ps = psS.tile([128, 1024], f32, tag="ps", name="pxs")
                            for qc in range(2):
                                nc.tensor.matmul(
                                    ps[:, qc * 512 : (qc + 1) * 512],
                                    qkk[ro : ro + 64, kc * 128 : (kc + 1) * 128],
                                    qkq[ro : ro + 64, qc * 512 : (qc + 1) * 512],
                                    start=True,
                                    stop=True,
                                )
                            pt = ptp.tile([128, 1024], bf16, tag="pt", name="pt")
                            # psum = (8q)(8k) = 64 * raw; true score = raw/8
                            nc.scalar.activation(
                                pt[:], ps[:], AF.Exp, scale=1.0 / (SQK * SQK * 8.0)
                            )
                            off = 1023 - kc * 128
                            nc.vector.tensor_mul(
                                pt[:], pt[:], rbs[odd][:, off : off + 1024]
                            )
                            ptiles[kc][odd] = pt
                        if hp + 1 < KT:
                            if kc == 2:
                                wqk_n = wqk_tiles.pop(hp + 1)
                                qkq_n = emit_qk(wqk_n, hp + 1, 0)
                            elif kc == 4:
                                qkk_n = emit_qk(wqk_n, hp + 1, 1)
                        # all v tiles must be emitted before the first AV uses
                        # them (tile deps follow emission order): 2 per odd kc
                        # during pair 0
                        if hp == 0 and kc % 2 == 1:
                            emit_v(range(2 * (kc // 2), 2 * (kc // 2) + 2))
                        if hp >= 1 and kc >= 5:
                            emit_avt_qt(hp - 1, prev_ptiles, kc - 5)
                    if hp >= 1:
                        for qt in range(3, NT):
                            emit_avt_qt(hp - 1, prev_ptiles, qt)
                    prev_ptiles = ptiles
                    if hp + 1 < KT:
                        qkq, qkk = qkq_n, qkk_n
                for qt in range(NT):
                    emit_avt_qt(KT - 1, prev_ptiles, qt)

            wq_scope.__exit__(None, None, None)

            # ---------------- phases E/F/H: proj+residual+LN2 then MLP per half
            h2T = [fm_tile(f"h2T{i}") for i in range(KT)]
            w2_scope = tc.tile_pool(name="w2p", bufs=1)
            w2p = w2_scope.__enter__()
            w1_all = w2p.tile([128, RT * C], bf16, tag="w1all")
            w2_all = w2p.tile([128, RT * C], bf16, tag="w2all")
            for r0 in range(0, RT, 6):
                cols = slice(r0 * C, (r0 + 6) * C)
                nc.sync.dma_start(w1_all[:, cols], wfc1_d[:, cols])
                nc.sync.dma_start(w2_all[:, cols], wfc2_d[:, cols])
            with (
                tc.tile_pool(name="h2p", bufs=6) as h2p,
                tc.tile_pool(name="grp", bufs=4) as grp,
                tc.tile_pool(name="o2p", bufs=13) as o2p,
                tc.tile_pool(name="obp", bufs=4) as obp,
                tc.tile_pool(name="ppsb", bufs=2) as ppsbp,
            ):

                def emit_projF(qc):
                    # proj token-major fp8 DoubleRow: stationary=aTp pair chunk,
                    # moving=wproj pair rows; psum = SA*SW*(out+bias)
                    wp3 = wp_all[:].rearrange("p (ct s) -> p ct s", s=C)
                    with tc.tile_pool(name="psP", bufs=2, space="PSUM") as psP:
                        for i in range(4):
                            t = qc * 4 + i
                            pp = psP.tile([128, C], f32, tag="pp", name=f"pp{t}")
                            for c0 in range(0, C, 512):
                                c1 = min(c0 + 512, C)
                                for cp in range(KT // 2):
                                    nc.tensor.matmul(
                                        pp[:, c0:c1],
                                        aTp[cp][:].rearrange(
                                            "p (two n) -> p two n", two=2
                                        )[:, :, t * 128 : (t + 1) * 128],
                                        wp3[:, 2 * cp : 2 * cp + 2, c0:c1],
                                        start=(cp == 0),
                                        stop=False,
                                        perf_mode=DR,
                                    )
                                nc.tensor.matmul(
                                    pp[:, c0:c1],
                                    onesP[:],
                                    bpr_row[:, c0:c1],
                                    start=False,
                                    stop=True,
                                )
                            # xs += pp / (SA*SW)
                            pp_sb = ppsbp.tile([128, C], f32, tag="ppsb",
                                               name=f"ppsb{t}")
                            nc.vector.tensor_scalar(
                                pp_sb[:], pp[:], 1.0 / (SA * SW), None, op0=OP.mult
                            )
                            nc.vector.tensor_add(
                                xs[:, t * C : (t + 1) * C],
                                xs[:, t * C : (t + 1) * C],
                                pp_sb[:],
                            )
                            h2 = h2p.tile([128, C], bf16, tag="h2", name=f"h2_{t}")
                            layernorm_to_bf16(h2[:], xs[:, t * C : (t + 1) * C])
                            for ct in range(KT):
                                ps = psP.tile(
                                    [128, 128], bf16, tag="tp", bufs=4, name="psf2"
                                )
                                nc.tensor.transpose(
                                    ps[:], h2[:, ct * 128 : (ct + 1) * 128], identb[:]
                                )
                                nc.scalar.copy(
                                    h2T[ct][:, t * 128 : (t + 1) * 128], ps[:]
                                )

                def emit_mlp(qc):
                    o2 = []
                    with tc.tile_pool(name="psO", bufs=6, space="PSUM") as ps_o:
                        pso = [
                            ps_o.tile([128, 512], f32, tag="pso", name=f"pso{qc}_{i}")
                            for i in range(KT)
                        ]
                        with tc.tile_pool(name="psG2", bufs=2, space="PSUM") as ps_g:
                            for r in range(RT):
                                psg = ps_g.tile([128, 512], f32, tag="psg", name="psg")
                                for ct in range(KT):
                                    nc.tensor.matmul(
                                        psg[:],
                                        w1_all[:, r * C + ct * 128 : r * C + (ct + 1) * 128],
                                        h2T[ct][:, qc * 512 : (qc + 1) * 512],
                                        start=(ct == 0),
                                        stop=(ct == KT - 1),
                                    )
                                gr = grp.tile([128, 512], bf16, tag="gr", name="gr")
                                nc.scalar.activation(
                                    gr[:], psg[:], AF.Gelu, bias=bfc1_sb[:, r : r + 1]
                                )
                                for co in range(KT):
                                    nc.tensor.matmul(
                                        pso[co][:],
                                        w2_all[:, r * C + co * 128 : r * C + (co + 1) * 128],
                                        gr[:],
                                        start=(r == 0),
                                        stop=(r == RT - 1),
                                    )
                        for co in range(KT):
                            o2t = o2p.tile(
                                [128, 512], bf16, tag="o2", name=f"o2_{qc}_{co}"
                            )
                            nc.scalar.activation(
                                o2t[:], pso[co][:], AF.Identity,
                                bias=bfc2_sb[:, co : co + 1],
                            )
                            o2.append(o2t)
                    return o2

                def emit_stores(qc, o2):
                    with tc.tile_pool(name="psH", bufs=2, space="PSUM") as psH:
                        for t4 in range(4):
                            t = qc * 4 + t4
                            ob = obp.tile([128, C], f32, tag="ob", name="ob")
                            for co in range(KT):
                                ps = psH.tile([128, 128], bf16, tag="tp", name="psh")
                                nc.tensor.transpose(
                                    ps[:], o2[co][:, t4 * 128 : (t4 + 1) * 128], identb[:]
                                )
                                nc.vector.tensor_add(
                                    ob[:, co * 128 : (co + 1) * 128],
                                    xs[:, t * C + co * 128 : t * C + (co + 1) * 128],
                                    ps[:],
                                )
                            nc.sync.dma_start(out_d[t * 128 : (t + 1) * 128, :], ob[:])

                emit_projF(0)
                o2_0 = emit_mlp(0)
                emit_projF(1)
                emit_stores(0, o2_0)
                o2_1 = emit_mlp(1)
                emit_stores(1, o2_1)
            w2_scope.__exit__(None, None, None)

    nc.compile()
    return nc


def _get_nc(reps=1):
    key = f"nc{reps}"
    if key not in _NC_CACHE:
        _NC_CACHE[key] = _build_nc(reps)
    return _NC_CACHE[key]


def _to_fp8(a, scale):
    import ml_dtypes

    x = np.asarray(a, dtype=np.float32) * scale
    x = np.clip(x, -240.0, 240.0)
    return x.astype(ml_dtypes.float8_e4m3fn)


def _host_prep(inputs):
    import ml_dtypes

    bf = ml_dtypes.bfloat16
    inp = {k: np.asarray(v) for k, v in inputs.items()}
    x = np.ascontiguousarray(inp["x"], dtype=np.float32)  # [8, 1024, 768]
    g1 = inp["ln1_g"].astype(np.float64)
    b1 = inp["ln1_b"].astype(np.float64)
    qkv_w = inp["qkv_w"].astype(np.float64)  # [2304, 768]
    # NOTE: no attention-scale folding into Wq (handled by exp scale)
    wqkvT = (qkv_w * g1[None, :]).T  # [768, 2304]
    # partition-major [128, KT*2304] fp8 x16
    wqkv_t = _to_fp8(
        np.ascontiguousarray(
            wqkvT.reshape(KT, 128, 3 * C).transpose(1, 0, 2).reshape(128, KT * 3 * C)
        ),
        SW,
    )
    bqkv = (qkv_w @ b1).astype(np.float32)  # [2304]

    wprojT = inp["proj_w"].astype(np.float32).T  # [768, 768]
    wproj_t = _to_fp8(
        np.ascontiguousarray(
            wprojT.reshape(KT, 128, C).transpose(1, 0, 2).reshape(128, KT * C)
        ),
        SW,
    )
    bproj = inp["proj_b"].astype(np.float32)

    g2 = inp["ln2_g"].astype(np.float64)
    b2 = inp["ln2_b"].astype(np.float64)
    fc1_w = inp["fc1_w"].astype(np.float64)  # [3072, 768]
    wfc1T = (fc1_w * g2[None, :]).T  # [768, 3072]
    wfc1_t = np.ascontiguousarray(
        wfc1T.reshape(KT, 128, RT, 128).transpose(1, 2, 0, 3).reshape(128, RT * C)
    ).astype(bf)
    bfc1 = (fc1_w @ b2 + inp["fc1_b"].astype(np.float64)).astype(np.float32)  # [3072]
    wfc2T = inp["fc2_w"].astype(np.float32).T  # [3072, 768]
    wfc2_t = np.ascontiguousarray(
        wfc2T.reshape(RT, 128, C).transpose(1, 0, 2).reshape(128, RT * C)
    ).astype(bf)
    bfc2 = inp["fc2_b"].astype(np.float32)

    # consts [128, 54]: bqkv qk x SQK (p-major 12 of 18), bfc1 24, bfc2 6
    consts = np.zeros((128, 54), np.float32)
    consts[:, 0:18] = (bqkv * SQK).reshape(18, 128).T
    consts[:, 24:48] = bfc1.reshape(24, 128).T
    consts[:, 48:54] = bfc2.reshape(6, 128).T
    # brows: [bqkv_v * SW, bproj * SA*SW] (compensated at drains)
    brows = np.stack(
        [bqkv[2 * C :] * SW, bproj * (SA * SW)]
    ).astype(bf)  # [2, C]

    # multiplicative rel-bias toeplitz band, bf16
    tab = np.exp(inp["rel_table"].astype(np.float64)).astype(np.float32)  # [129, 12]
    p_i = np.arange(128)
    w_i = np.arange(W)
    idx = np.clip(p_i[:, None] + (N + 63) - w_i[None, :], 0, 2 * 64)
    rband = np.ascontiguousarray(
        tab[idx, :].transpose(0, 2, 1).reshape(128, H * W)
    ).astype(bf)

    shared = {
        "wqkv_t": wqkv_t,
        "wproj_t": wproj_t,
        "wfc1_t": wfc1_t,
        "wfc2_t": wfc2_t,
        "consts": consts,
        "brows": brows,
        "rband": rband,
    }
    in_maps = [{"x": np.ascontiguousarray(x[c]), **shared} for c in range(B)]
    return in_maps


def _make_runner(reps=1):
    import jax
    from jax.experimental.shard_map import shard_map
    from jax.sharding import Mesh, NamedSharding, PartitionSpec

    from concourse import bass2jax, mybir

    nc = _get_nc(reps)
    bass2jax.install_neuronx_cc_hook()

    partition_name = nc.partition_id_tensor.name if nc.partition_id_tensor else None
    in_names, out_names, out_avals, zero_outs = [], [], [], []
    for alloc in nc.m.functions[0].allocations:
        if not isinstance(alloc, mybir.MemoryLocationSet):
            continue
        name = alloc.memorylocations[0].name
        if alloc.kind == "ExternalInput":
            if name != partition_name:
                in_names.append(name)
        elif alloc.kind == "ExternalOutput":
            out_names.append(name)
            shape = tuple(alloc.tensor_shape)
            dtype = mybir.dt.np(alloc.dtype)
            out_avals.append(jax.core.ShapedArray(shape, dtype))
            zero_outs.append(np.zeros(shape, dtype))
    n_params = len(in_names)
    all_names = tuple(in_names) + tuple(out_names)
    if partition_name is not None:
        all_names = all_names + (partition_name,)
    donate = tuple(range(n_params, n_params + len(out_names)))

    def _body(*args):
        operands = list(args)
        if partition_name is not None:
            operands.append(bass2jax.partition_id_tensor())
        outs = bass2jax._bass_exec_p.bind(
            *operands,
            out_avals=tuple(out_avals),
            in_names=all_names,
            out_names=tuple(out_names),
            lowering_input_output_aliases=(),
            sim_require_finite=True,
            sim_require_nnan=True,
            nc=nc,
        )
        return tuple(outs)

    def _body_k(k):
        def body(*args):
            ins = list(args[:n_params])
            outs = list(args[n_params:])
            for _ in range(k):
                outs = list(_body(*ins, *outs))
            return tuple(outs)

        return body

    devices = jax.devices()[:B]
    mesh = Mesh(np.asarray(devices), ("core",))
    in_specs = (PartitionSpec("core"),) * (n_params + len(out_names))
    out_specs = (PartitionSpec("core"),) * len(out_names)

    def make_fn(k):
        return jax.jit(
            shard_map(
                _body_k(k),
                mesh=mesh,
                in_specs=in_specs,
                out_specs=out_specs,
                check_rep=False,
            ),
            donate_argnums=donate,
            keep_unused=True,
        )

    sharding = NamedSharding(mesh, PartitionSpec("core"))
    return make_fn, in_names, out_names, zero_outs, sharding


def _get_runner(reps=1):
    key = f"runner{reps}"
    if key not in _NC_CACHE:
        _NC_CACHE[key] = _make_runner(reps)
    return _NC_CACHE[key]


LAST_BENCH = None


def kernel(**inputs):
    global LAST_BENCH
    import time

    import jax

    make_fn, in_names, out_names, zero_outs, sharding = _get_runner()
    in_maps = _host_prep(inputs)
    concat_in = [
        np.concatenate([np.asarray(in_maps[c][n]) for c in range(B)], axis=0)
        for n in in_names
    ]
    concat_zeros = [
        np.zeros((B * z.shape[0], *z.shape[1:]), z.dtype) for z in zero_outs
    ]
    fn1 = make_fn(1)
    dev_in = [jax.device_put(a, sharding) for a in concat_in]
    outs = fn1(*dev_in, *concat_zeros)
    jax.block_until_ready(outs)
    result = np.asarray(outs[0]).reshape(B, N, C).astype(np.float32)
    return result


# revision 3
# speedup vs baseline: 1.2828x; 1.2204x over previous
"""Trainium2 Bass kernel for a dense transformer block (B=8, N=1024, C=768, H=12).

Sharding: data-parallel over batch -- one batch element per NeuronCore (8 cores),
weights replicated, no collectives.

v2: attention matmuls in fp8e4m3 with DoubleRow (double-pumped) mode:
  - hT stored fp8 in ct-pair layout [128, 2, 1024] -> QKV matmuls DoubleRow
  - q/k stored fp8 (x8 scale), scores matmul fp8 (contraction 64)
  - exp compensates scales via activation scale=1/512
  - v/pt/AV stay bf16 (keeps DVE 2x for the band multiply)
  - aT stored fp8 (x8) in hp-pair layout -> proj matmul DoubleRow (w x16)
  - bias rank-1 matmuls in bf16 (were fp32: 4 cycles/row)
v3: MLP also fp8 DoubleRow (h2T/gr fp8, fc1/fc2 weights fp8 x16).
"""

import os

import numpy as np

B, N, C, H, D = 8, 1024, 768, 12, 64
NT = N // 128   # 8 token tiles
KT = C // 128   # 6 feature tiles
F1 = 4 * C      # 3072
RT = F1 // 128  # 24
W = 2 * N - 1   # 2047 toeplitz band width
EPS = 1e-5

SW = 16.0       # fp8 weight pre-scale
SQK = 8.0       # fp8 q/k storage scale
SA = 8.0        # fp8 aT storage scale

LAST_RESULTS = None

_NC_CACHE = {}


def _build_nc(reps=1):
    from contextlib import ExitStack

    import concourse.bacc as bacc
    import concourse.tile as tile
    from concourse import masks, mybir

    f32 = mybir.dt.float32
    bf16 = mybir.dt.bfloat16
    f8 = mybir.dt.float8e4

    AF = mybir.ActivationFunctionType
    OP = mybir.AluOpType
    DR = mybir.MatmulPerfMode.DoubleRow

    nc = bacc.Bacc(
        "TRN2",
        target_bir_lowering=False,
        debug=False,
        enable_asserts=False,
        num_devices=8,
    )

    x_d = nc.dram_tensor("x", [N, C], f32, kind="ExternalInput").ap()
    wqkv_d = nc.dram_tensor("wqkv_t", [128, KT * 3 * C], f8, kind="ExternalInput").ap()
    wproj_d = nc.dram_tensor("wproj_t", [128, KT * C], f8, kind="ExternalInput").ap()
    wfc1_d = nc.dram_tensor("wfc1_t", [128, RT * C], f8, kind="ExternalInput").ap()
    wfc2_d = nc.dram_tensor("wfc2_t", [128, RT * C], f8, kind="ExternalInput").ap()
    consts_d = nc.dram_tensor("consts", [128, 54], f32, kind="ExternalInput").ap()
    brows_d = nc.dram_tensor("brows", [2, C], bf16, kind="ExternalInput").ap()
    rb_d = nc.dram_tensor("rband", [128, H * W], bf16, kind="ExternalInput").ap()
    out_d = nc.dram_tensor("out", [N, C], f32, kind="ExternalOutput").ap()

    with tile.TileContext(nc) as tc, ExitStack() as ctx:
        # ---------------- kernel-wide pools
        cpool = ctx.enter_context(tc.tile_pool(name="const", bufs=1))
        identb = cpool.tile([128, 128], bf16, tag="identb")
        masks.make_identity(nc, identb[:])
        onesP = cpool.tile([1, 128], bf16, tag="onesP")
        nc.any.memset(onesP[:], 1.0)
        # consts layout: 0:12 bqkv(qk, x8), 18:24 bproj(unused; brows carries),
        # 24:48 bfc1, 48:54 bfc2
        consts = cpool.tile([128, 54], f32, tag="consts")
        bqkv_sb = consts[:, 0:18]
        bfc1_sb = consts[:, 24:48]
        bfc2_sb = consts[:, 48:54]
        bv_row = cpool.tile([1, C], bf16, tag="bvrow")
        bpr_row = cpool.tile([1, C], bf16, tag="bprrow")

        # persistent x (residual stream), f32 [128, 8*768]
        xs = cpool.tile([128, NT * C], f32, tag="xs")

        stat = ctx.enter_context(tc.tile_pool(name="stat", bufs=8))
        chain = ctx.enter_context(tc.tile_pool(name="chain", bufs=1))

        def fm_tile(name):
            return chain.tile([128, N], bf16, tag="fm1024", bufs=6, name=name)

        def pair_tile(name):
            # fp8 ct-pair layout: [128, 2*1024]
            return chain.tile([128, 2 * N], f8, tag="pair2048", bufs=9, name=name)

        def pview(t):
            return t[:].rearrange("p (two n) -> p two n", two=2)

        epsc = cpool.tile([128, 1], f32, tag="eps")
        nc.any.memset(epsc[:], EPS)

        def layernorm_to_bf16(dst_ap, src_ap):
            """dst(bf16) = (src - mean) * rsqrt(var + eps); src [128, 768] f32."""
            st6 = stat.tile([128, 12], f32, tag="st6", name="st6")
            nc.vector.bn_stats(st6[:, 0:6], src_ap[:, 0:384])
            nc.vector.bn_stats(st6[:, 6:12], src_ap[:, 384:768])
            mv = stat.tile([128, 2], f32, tag="mv", name="mv")
            nc.vector.bn_aggr(mv[:], st6[:])
            sd = stat.tile([128, 1], f32, tag="sd", name="sd")
            nc.scalar.activation(sd[:], mv[:, 1:2], AF.Sqrt, bias=epsc[:])
            rstd = stat.tile([128, 1], f32, tag="rstd", name="rstd")
            nc.vector.reciprocal(rstd[:], sd[:])
            nmrp = stat.tile([128, 1], f32, tag="nmrp", name="nmrp")
            nc.vector.tensor_mul(nmrp[:], mv[:, 0:1], rstd[:])
            nc.vector.tensor_scalar(
                dst_ap, src_ap, rstd[:], nmrp[:], op0=OP.mult, op1=OP.subtract
            )

        for _rep in range(reps):
            # ---------------- phase A: load x, LN1, transpose -> hTp (fp8 pairs)
            hTp = [pair_tile(f"hTp{i}") for i in range(KT // 2)]

            def hT_view(i):
                return hTp[i][:].rearrange("p (two n) -> p two n", two=2)

            wq_scope = tc.tile_pool(name="wqp", bufs=1)
            wqp = wq_scope.__enter__()
            with (
                tc.tile_pool(name="h1p", bufs=6) as h1p,
                tc.tile_pool(name="psA", bufs=8, space="PSUM") as psA,
            ):
                for t in range(NT):
                    nc.sync.dma_start(
                        xs[:, t * C : (t + 1) * C], x_d[t * 128 : (t + 1) * 128, :]
                    )
                nc.sync.dma_start(consts[:], consts_d[:])
                nc.sync.dma_start(bv_row[:], brows_d[0:1, :])
                nc.sync.dma_start(bpr_row[:], brows_d[1:2, :])
                wqkv3 = wqkv_d[:].rearrange("p (ct s) -> p ct s", s=2304)
                wqk_tiles = {}

                def load_wqk(hp):
                    wqk = wqp.tile([128, KT * 256], f8, tag="wqk", bufs=2,
                                   name=f"wqk{hp}")
                    wqk3 = wqk[:].rearrange("p (ct s) -> p ct s", s=256)
                    nc.sync.dma_start(
                        wqk3[:, :, 0:128], wqkv3[:, :, hp * 128 : (hp + 1) * 128]
                    )
                    nc.sync.dma_start(
                        wqk3[:, :, 128:256],
                        wqkv3[:, :, C + hp * 128 : C + (hp + 1) * 128],
                    )
                    wqk_tiles[hp] = wqk

                load_wqk(0)
                wv = wqp.tile([128, KT * C], f8, tag="wv")
                nc.sync.dma_start(
                    wv[:].rearrange("p (ct s) -> p ct s", s=C),
                    wqkv3[:, :, 2 * C : 3 * C],
                )
                wp_all = cpool.tile([128, KT * C], f8, tag="wpall")
                for t in range(NT):
                    h1 = h1p.tile([128, C], bf16, tag="h1", name=f"h1_{t}")
                    layernorm_to_bf16(h1[:], xs[:, t * C : (t + 1) * C])
                    for ct in range(KT):
                        ps = psA.tile([128, 128], bf16, tag="tp", name="psa")
                        nc.tensor.transpose(
                            ps[:], h1[:, ct * 128 : (ct + 1) * 128], identb[:]
                        )
                        dst = hTp[ct // 2][
                            :, (ct % 2) * N + t * 128 : (ct % 2) * N + (t + 1) * 128
                        ]
                        if ct % 2:
                            nc.scalar.copy(dst, ps[:])
                        else:
                            nc.vector.tensor_copy(dst, ps[:])

            # ---------------- phases C+D fused: QKV + attention, per head pair.
            aTp = [pair_tile(f"aTp{i}") for i in range(KT // 2)]
            vaug = [
                chain.tile([128, H * 65], bf16, tag="vaug", bufs=NT, name=f"vaug{t}")
                for t in range(NT)
            ]
            with (
                tc.tile_pool(name="qkp", bufs=4) as qkp,
                tc.tile_pool(name="rbp", bufs=4) as rbp,
                tc.tile_pool(name="ptp", bufs=40) as ptp,
                tc.tile_pool(name="atokp", bufs=4) as atokp,
                tc.tile_pool(name="psS", bufs=2, space="PSUM") as psS,
                tc.tile_pool(name="psM", bufs=2, space="PSUM") as psM,
                tc.tile_pool(name="psT", bufs=2, space="PSUM") as psT,
            ):

                def emit_qk(wqk, hp, sec):
                    # out fp8 q or k tile [128, N] = 8*(raw + bias)
                    jt = hp + 6 * sec
                    qk = qkp.tile([128, N], f8, tag="qk", name=f"qk{jt}")
                    wqk3 = wqk[:].rearrange("p (ct s) -> p ct s", s=256)
                    for qc in range(2):
                        px = psM.tile([128, 512], f32, tag="pm", name="pxq")
                        ps = px[:]
                        for cp in range(KT // 2):
                            nc.tensor.matmul(
                                ps,
                                wqk3[
                                    :, 2 * cp : 2 * cp + 2,
                                    sec * 128 : (sec + 1) * 128,
                                ],
                                hT_view(cp)[:, :, qc * 512 : (qc + 1) * 512],
                                start=(cp == 0),
                                stop=(cp == KT // 2 - 1),
                                perf_mode=DR,
                            )
                        # qk_f8 = ps * (SQK/SW) + bias8 (bias pre-scaled x8 host)
                        nc.vector.tensor_scalar(
                            qk[:, qc * 512 : (qc + 1) * 512],
                            ps,
                            SQK / SW,
                            bqkv_sb[:, jt : jt + 1],
                            op0=OP.mult,
                            op1=OP.add,
                        )
                    return qk

                def emit_v(trange):
                    for t in trange:
                        vview = vaug[t][:].rearrange("p (h e) -> p h e", e=65)
                        for vc in range(2):
                            px = psM.tile([128, 512], f32, tag="pm", name="pxv")
                            ps = px[:, 0:384]
                            wv3 = wv[:].rearrange("p (ct s) -> p ct s", s=C)
                            for cp in range(KT // 2):
                                nc.tensor.matmul(
                                    ps,
                                    hT_view(cp)[:, :, t * 128 : (t + 1) * 128],
                                    wv3[
                                        :, 2 * cp : 2 * cp + 2,
                                        vc * 384 : (vc + 1) * 384,
                                    ],
                                    start=(cp == 0),
                                    stop=False,
                                    perf_mode=DR,
                                )
                            nc.tensor.matmul(
                                ps,
                                onesP[:],
                                bv_row[:, vc * 384 : (vc + 1) * 384],
                                start=False,
                                stop=True,
                            )
                            # v = ps / SW  (bias row pre-scaled x16 host)
                            nc.vector.tensor_scalar(
                                vview[:, vc * 6 : (vc + 1) * 6, 0:64],
                                ps.rearrange("p (h e) -> p h e", e=64),
                                1.0 / SW,
                                None,
                                op0=OP.mult,
                            )
                        nc.any.memset(vview[:, :, 64:65], 1.0)

                def emit_avt_qt(hp, ptiles, qt):
                        px = psM.tile([128, 512], f32, tag="pm", name="pxa")
                        psq = px[:, 0:130]
                        for kc in range(NT):
                            for odd in range(2):
                                h = 2 * hp + odd
                                nc.tensor.matmul(
                                    psq[:, odd * 65 : (odd + 1) * 65],
                                    ptiles[kc][odd][:, qt * 128 : (qt + 1) * 128],
                                    vaug[kc][:, h * 65 : (h + 1) * 65],
                                    start=(kc == 0 and odd == 0),
                                    stop=(kc == NT - 1 and odd == 1),
                                )
                        rec = stat.tile([128, 2], f32, tag="rec", name="rec")
                        nc.vector.reciprocal(rec[:, 0:1], psq[:, 64:65])
                        nc.vector.reciprocal(rec[:, 1:2], psq[:, 129:130])
                        atok = atokp.tile([128, 128], bf16, tag="atok", name="atok")
                        # atok = (psq * rec) * SA  (fp8 storage scale)
                        nc.vector.tensor_scalar(
                            atok[:, 0:64], psq[:, 0:64], rec[:, 0:1], SA,
                            op0=OP.mult, op1=OP.mult,
                        )
                        nc.vector.tensor_scalar(
                            atok[:, 64:128], psq[:, 65:129], rec[:, 1:2], SA,
                            op0=OP.mult, op1=OP.mult,
                        )
                        pst = psT.tile([128, 128], bf16, tag="pst", name="pst")
                        nc.tensor.transpose(pst[:], atok[:], identb[:])
                        dst = aTp[hp // 2][
                            :, (hp % 2) * N + qt * 128 : (hp % 2) * N + (qt + 1) * 128
                        ]
                        nc.vector.tensor_copy(dst, pst[:])

                prev_ptiles = None
                for hp in range(KT):
                    if hp + 1 < KT:
                        load_wqk(hp + 1)
                    if hp == 4:
                        nc.sync.dma_start(wp_all[:], wproj_d[:])
                    if hp == 0:
                        wqk = wqk_tiles.pop(0)
                        qkq = emit_qk(wqk, 0, 0)
                        qkk = emit_qk(wqk, 0, 1)
                    rbs = []
                    for odd in range(2):
                        rbh = rbp.tile([128, W], bf16, tag="rb", name=f"rb{2*hp+odd}")
                        nc.sync.dma_start(
                            rbh[:],
                            rb_d[:, (2 * hp + odd) * W : (2 * hp + odd + 1) * W],
                        )
                        rbs.append(rbh)
                    ptiles = [[None] * 2 for _ in range(NT)]
                    for kc in range(NT):
                        for odd in range(2):
                            ro = odd * 64
                            ps = psS.tile([128, 1024], f32, tag="ps", name="pxs")
                            for qc in range(2):
                                nc.tensor.matmul(
                                    ps[:, qc * 512 : (qc + 1) * 512],
                                    qkk[ro : ro + 64, kc * 128 : (kc + 1) * 128],
                                    qkq[ro : ro + 64, qc * 512 : (qc + 1) * 512],
                                    start=True,
                                    stop=True,
                                )
                            pt = ptp.tile([128, 1024], bf16, tag="pt", name="pt")
                            # psum = (8q)(8k) = 64 * raw; true score = raw/8
                            nc.scalar.activation(
                                pt[:], ps[:], AF.Exp, scale=1.0 / (SQK * SQK * 8.0)
                            )
                            off = 1023 - kc * 128
                            nc.vector.tensor_mul(
                                pt[:], pt[:], rbs[odd][:, off : off + 1024]
                            )
                            ptiles[kc][odd] = pt
                        if hp + 1 < KT:
                            if kc == 2:
                                wqk_n = wqk_tiles.pop(hp + 1)
                                qkq_n = emit_qk(wqk_n, hp + 1, 0)
                            elif kc == 4:
                                qkk_n = emit_qk(wqk_n, hp + 1, 1)
                        # all v tiles must be emitted before the first AV uses
                        # them (tile deps follow emission order): 2 per odd kc
                        # during pair 0
                        if hp == 0 and kc % 2 == 1:
                            emit_v(range(2 * (kc // 2), 2 * (kc // 2) + 2))
                        if hp >= 1 and kc >= 5:
                            emit_avt_qt(hp - 1, prev_ptiles, kc - 5)
                    if hp >= 1:
                        for qt in range(3, NT):
                            emit_avt_qt(hp - 1, prev_ptiles, qt)
                    prev_ptiles = ptiles
                    if hp + 1 < KT:
                        qkq, qkk = qkq_n, qkk_n
                for qt in range(NT):
                    emit_avt_qt(KT - 1, prev_ptiles, qt)

            wq_scope.__exit__(None, None, None)

            # ---------------- phases E/F/H: proj+residual+LN2 then MLP per half
            h2Tp = [pair_tile(f"h2Tp{i}") for i in range(KT // 2)]
            w2_scope = tc.tile_pool(name="w2p", bufs=1)
            w2p = w2_scope.__enter__()
            w1_all = w2p.tile([128, RT * C], f8, tag="w1all")
            w2_all = w2p.tile([128, RT * C], f8, tag="w2all")
            for r0 in range(0, RT, 6):
                cols = slice(r0 * C, (r0 + 6) * C)
                nc.sync.dma_start(w1_all[:, cols], wfc1_d[:, cols])
                nc.sync.dma_start(w2_all[:, cols], wfc2_d[:, cols])
            with (
                tc.tile_pool(name="h2p", bufs=6) as h2p,
                tc.tile_pool(name="grp", bufs=4) as grp,
                tc.tile_pool(name="o2p", bufs=13) as o2p,
                tc.tile_pool(name="obp", bufs=4) as obp,
                tc.tile_pool(name="ppsb", bufs=2) as ppsbp,
            ):

                def emit_projF(qc):
                    # proj token-major fp8 DoubleRow: stationary=aTp pair chunk,
                    # moving=wproj pair rows; psum = SA*SW*(out+bias)
                    wp3 = wp_all[:].rearrange("p (ct s) -> p ct s", s=C)
                    with tc.tile_pool(name="psP", bufs=2, space="PSUM") as psP:
                        for i in range(4):
                            t = qc * 4 + i
                            pp = psP.tile([128, C], f32, tag="pp", name=f"pp{t}")
                            for c0 in range(0, C, 512):
                                c1 = min(c0 + 512, C)
                                for cp in range(KT // 2):
                                    nc.tensor.matmul(
                                        pp[:, c0:c1],
                                        aTp[cp][:].rearrange(
                                            "p (two n) -> p two n", two=2
                                        )[:, :, t * 128 : (t + 1) * 128],
                                        wp3[:, 2 * cp : 2 * cp + 2, c0:c1],
                                        start=(cp == 0),
                                        stop=False,
                                        perf_mode=DR,
                                    )
                                nc.tensor.matmul(
                                    pp[:, c0:c1],
                                    onesP[:],
                                    bpr_row[:, c0:c1],
                                    start=False,
                                    stop=True,
                                )
                            # xs += pp / (SA*SW)
                            pp_sb = ppsbp.tile([128, C], f32, tag="ppsb",
                                               name=f"ppsb{t}")
                            nc.vector.tensor_scalar(
                                pp_sb[:], pp[:], 1.0 / (SA * SW), None, op0=OP.mult
                            )
                            nc.vector.tensor_add(
                                xs[:, t * C : (t + 1) * C],
                                xs[:, t * C : (t + 1) * C],
                                pp_sb[:],
                            )
                            h2 = h2p.tile([128, C], bf16, tag="h2", name=f"h2_{t}")
                            layernorm_to_bf16(h2[:], xs[:, t * C : (t + 1) * C])
                            for ct in range(KT):
                                ps = psP.tile(
                                    [128, 128], bf16, tag="tp", bufs=4, name="psf2"
                                )
                                nc.tensor.transpose(
                                    ps[:], h2[:, ct * 128 : (ct + 1) * 128], identb[:]
                                )
                                dst = h2Tp[ct // 2][
                                    :,
                                    (ct % 2) * N + t * 128 : (ct % 2) * N + (t + 1) * 128,
                                ]
                                nc.scalar.copy(dst, ps[:])

                def emit_mlp(qc):
                    # fc1/fc2 fp8 DoubleRow: psg = SW*(h2@W1); gr fp8 = gelu
                    # r-pair tiles (even half 0:512, odd half 512:1024) so fc2
                    # can pair the contraction over r.
                    o2 = []
                    w1v = w1_all[:].rearrange("p (r c) -> p r c", c=C)
                    w2v = w2_all[:].rearrange("p (r c) -> p r c", c=C)
                    with tc.tile_pool(name="psO", bufs=6, space="PSUM") as ps_o:
                        pso = [
                            ps_o.tile([128, 512], f32, tag="pso", name=f"pso{qc}_{i}")
                            for i in range(KT)
                        ]
                        with tc.tile_pool(name="psG2", bufs=2, space="PSUM") as ps_g:
                            for rp in range(RT // 2):
                                grt = grp.tile(
                                    [128, 1024], f8, tag="gr", name=f"gr{rp}"
                                )
                                for half in range(2):
                                    r = 2 * rp + half
                                    psg = ps_g.tile(
                                        [128, 512], f32, tag="psg", name="psg"
                                    )
                                    for cp in range(KT // 2):
                                        nc.tensor.matmul(
                                            psg[:],
                                            w1v[
                                                :, r,
                                                2 * cp * 128 : (2 * cp + 2) * 128,
                                            ].rearrange(
                                                "p (two j) -> p two j", two=2
                                            ),
                                            pview(h2Tp[cp])[
                                                :, :, qc * 512 : (qc + 1) * 512
                                            ],
                                            start=(cp == 0),
                                            stop=(cp == KT // 2 - 1),
                                            perf_mode=DR,
                                        )
                                    # gr = gelu(psg/SW + b); fp8 storage
                                    nc.scalar.activation(
                                        grt[:, half * 512 : (half + 1) * 512],
                                        psg[:],
                                        AF.Gelu,
                                        bias=bfc1_sb[:, r : r + 1],
                                        scale=1.0 / SW,
                                    )
                                grv = grt[:].rearrange("p (two c) -> p two c", two=2)
                                for co in range(KT):
                                    nc.tensor.matmul(
                                        pso[co][:],
                                        w2v[
                                            :, 2 * rp : 2 * rp + 2,
                                            co * 128 : (co + 1) * 128,
                                        ],
                                        grv,
                                        start=(rp == 0),
                                        stop=(rp == RT // 2 - 1),
                                        perf_mode=DR,
                                    )
                        for co in range(KT):
                            o2t = o2p.tile(
                                [128, 512], bf16, tag="o2", name=f"o2_{qc}_{co}"
                            )
                            nc.scalar.activation(
                                o2t[:], pso[co][:], AF.Identity,
                                bias=bfc2_sb[:, co : co + 1],
                                scale=1.0 / SW,
                            )
                            o2.append(o2t)
                    return o2

                def emit_stores(qc, o2):
                    with tc.tile_pool(name="psH", bufs=2, space="PSUM") as psH:
                        for t4 in range(4):
                            t = qc * 4 + t4
                            ob = obp.tile([128, C], f32, tag="ob", name="ob")
                            for co in range(KT):
                                ps = psH.tile([128, 128], bf16, tag="tp", name="psh")
                                nc.tensor.transpose(
                                    ps[:], o2[co][:, t4 * 128 : (t4 + 1) * 128], identb[:]
                                )
                                nc.vector.tensor_add(
                                    ob[:, co * 128 : (co + 1) * 128],
                                    xs[:, t * C + co * 128 : t * C + (co + 1) * 128],
                                    ps[:],
                                )
                            nc.sync.dma_start(out_d[t * 128 : (t + 1) * 128, :], ob[:])

                emit_projF(0)
                o2_0 = emit_mlp(0)
                emit_projF(1)
                emit_stores(0, o2_0)
                o2_1 = emit_mlp(1)
                emit_stores(1, o2_1)
            w2_scope.__exit__(None, None, None)

    nc.compile()
    return nc


def _get_nc(reps=1):
    key = f"nc{reps}"
    if key not in _NC_CACHE:
        _NC_CACHE[key] = _build_nc(reps)
    return _NC_CACHE[key]


def _to_fp8(a, scale):
    import ml_dtypes

    x = np.asarray(a, dtype=np.float32) * scale
    x = np.clip(x, -240.0, 240.0)
    return x.astype(ml_dtypes.float8_e4m3fn)


def _host_prep(inputs):
    import ml_dtypes

    bf = ml_dtypes.bfloat16
    inp = {k: np.asarray(v) for k, v in inputs.items()}
    x = np.ascontiguousarray(inp["x"], dtype=np.float32)  # [8, 1024, 768]
    g1 = inp["ln1_g"].astype(np.float64)
    b1 = inp["ln1_b"].astype(np.float64)
    qkv_w = inp["qkv_w"].astype(np.float64)  # [2304, 768]
    # NOTE: no attention-scale folding into Wq (handled by exp scale)
    wqkvT = (qkv_w * g1[None, :]).T  # [768, 2304]
    # partition-major [128, KT*2304] fp8 x16
    wqkv_t = _to_fp8(
        np.ascontiguousarray(
            wqkvT.reshape(KT, 128, 3 * C).transpose(1, 0, 2).reshape(128, KT * 3 * C)
        ),
        SW,
    )
    bqkv = (qkv_w @ b1).astype(np.float32)  # [2304]

    wprojT = inp["proj_w"].astype(np.float32).T  # [768, 768]
    wproj_t = _to_fp8(
        np.ascontiguousarray(
            wprojT.reshape(KT, 128, C).transpose(1, 0, 2).reshape(128, KT * C)
        ),
        SW,
    )
    bproj = inp["proj_b"].astype(np.float32)

    g2 = inp["ln2_g"].astype(np.float64)
    b2 = inp["ln2_b"].astype(np.float64)
    fc1_w = inp["fc1_w"].astype(np.float64)  # [3072, 768]
    wfc1T = (fc1_w * g2[None, :]).T  # [768, 3072]
    wfc1_t = _to_fp8(
        np.ascontiguousarray(
            wfc1T.reshape(KT, 128, RT, 128).transpose(1, 2, 0, 3).reshape(128, RT * C)
        ),
        SW,
    )
    bfc1 = (fc1_w @ b2 + inp["fc1_b"].astype(np.float64)).astype(np.float32)  # [3072]
    wfc2T = inp["fc2_w"].astype(np.float32).T  # [3072, 768]
    wfc2_t = _to_fp8(
        np.ascontiguousarray(
            wfc2T.reshape(RT, 128, C).transpose(1, 0, 2).reshape(128, RT * C)
        ),
        SW,
    )
    bfc2 = inp["fc2_b"].astype(np.float32)

    # consts [128, 54]: bqkv qk x SQK (p-major 12 of 18), bfc1 24, bfc2 6
    consts = np.zeros((128, 54), np.float32)
    consts[:, 0:18] = (bqkv * SQK).reshape(18, 128).T
    consts[:, 24:48] = bfc1.reshape(24, 128).T
    consts[:, 48:54] = bfc2.reshape(6, 128).T
    # brows: [bqkv_v * SW, bproj * SA*SW] (compensated at drains)
    brows = np.stack(
        [bqkv[2 * C :] * SW, bproj * (SA * SW)]
    ).astype(bf)  # [2, C]

    # multiplicative rel-bias toeplitz band, bf16
    tab = np.exp(inp["rel_table"].astype(np.float64)).astype(np.float32)  # [129, 12]
    p_i = np.arange(128)
    w_i = np.arange(W)
    idx = np.clip(p_i[:, None] + (N + 63) - w_i[None, :], 0, 2 * 64)
    rband = np.ascontiguousarray(
        tab[idx, :].transpose(0, 2, 1).reshape(128, H * W)
    ).astype(bf)

    shared = {
        "wqkv_t": wqkv_t,
        "wproj_t": wproj_t,
        "wfc1_t": wfc1_t,
        "wfc2_t": wfc2_t,
        "consts": consts,
        "brows": brows,
        "rband": rband,
    }
    in_maps = [{"x": np.ascontiguousarray(x[c]), **shared} for c in range(B)]
    return in_maps


def _make_runner(reps=1):
    import jax
    from jax.experimental.shard_map import shard_map
    from jax.sharding import Mesh, NamedSharding, PartitionSpec

    from concourse import bass2jax, mybir

    nc = _get_nc(reps)
    bass2jax.install_neuronx_cc_hook()

    partition_name = nc.partition_id_tensor.name if nc.partition_id_tensor else None
    in_names, out_names, out_avals, zero_outs = [], [], [], []
    for alloc in nc.m.functions[0].allocations:
        if not isinstance(alloc, mybir.MemoryLocationSet):
            continue
        name = alloc.memorylocations[0].name
        if alloc.kind == "ExternalInput":
            if name != partition_name:
                in_names.append(name)
        elif alloc.kind == "ExternalOutput":
            out_names.append(name)
            shape = tuple(alloc.tensor_shape)
            dtype = mybir.dt.np(alloc.dtype)
            out_avals.append(jax.core.ShapedArray(shape, dtype))
            zero_outs.append(np.zeros(shape, dtype))
    n_params = len(in_names)
    all_names = tuple(in_names) + tuple(out_names)
    if partition_name is not None:
        all_names = all_names + (partition_name,)
    donate = tuple(range(n_params, n_params + len(out_names)))

    def _body(*args):
        operands = list(args)
        if partition_name is not None:
            operands.append(bass2jax.partition_id_tensor())
        outs = bass2jax._bass_exec_p.bind(
            *operands,
            out_avals=tuple(out_avals),
            in_names=all_names,
            out_names=tuple(out_names),
            lowering_input_output_aliases=(),
            sim_require_finite=True,
            sim_require_nnan=True,
            nc=nc,
        )
        return tuple(outs)

    def _body_k(k):
        def body(*args):
            ins = list(args[:n_params])
            outs = list(args[n_params:])
            for _ in range(k):
                outs = list(_body(*ins, *outs))
            return tuple(outs)

        return body

    devices = jax.devices()[:B]
    mesh = Mesh(np.asarray(devices), ("core",))
    in_specs = (PartitionSpec("core"),) * (n_params + len(out_names))
    out_specs = (PartitionSpec("core"),) * len(out_names)

    def make_fn(k):
        return jax.jit(
            shard_map(
                _body_k(k),
                mesh=mesh,
                in_specs=in_specs,
                out_specs=out_specs,
                check_rep=False,
            ),
            donate_argnums=donate,
            keep_unused=True,
        )

    sharding = NamedSharding(mesh, PartitionSpec("core"))
    return make_fn, in_names, out_names, zero_outs, sharding


def _get_runner(reps=1):
    key = f"runner{reps}"
    if key not in _NC_CACHE:
        _NC_CACHE[key] = _make_runner(reps)
    return _NC_CACHE[key]


LAST_BENCH = None


def kernel(**inputs):
    global LAST_BENCH
    import time

    import jax

    make_fn, in_names, out_names, zero_outs, sharding = _get_runner()
    in_maps = _host_prep(inputs)
    concat_in = [
        np.concatenate([np.asarray(in_maps[c][n]) for c in range(B)], axis=0)
        for n in in_names
    ]
    concat_zeros = [
        np.zeros((B * z.shape[0], *z.shape[1:]), z.dtype) for z in zero_outs
    ]
    fn1 = make_fn(1)
    dev_in = [jax.device_put(a, sharding) for a in concat_in]
    outs = fn1(*dev_in, *concat_zeros)
    jax.block_until_ready(outs)
    result = np.asarray(outs[0]).reshape(B, N, C).astype(np.float32)
    return result
